# revision 1
# baseline (speedup 1.0000x reference)
"""CPGNN (compatibility-guided GNN) kernel for 8 Trainium2 NeuronCores.

Reference computation (N=10000, F=512, HID=256, C=16, 4 post iterations):
    h      = relu(normed_adj @ (features @ W1) + b1)
    logits = normed_adj @ (h @ W2) + b2
    E_hat  = softmax(logits) - 1/C
    B_hat  = E_hat;  4x: B_hat = E_hat + raw_adj @ (B_hat @ H)
    out    = B_hat + 1/C

Sharding: rows of both adjacency matrices are sharded over the 8 cores
(1280 rows per core, tail core padded).  The adjacency shards are
uploaded TRANSPOSED (K-major, [10240, 1280] bf16) so every on-device
matmul can use natural layouts.  The small per-core [rows, C] matrices
are all-gathered between phases via device collectives.  All big
matmuls run in bf16 with fp32 PSUM accumulation (verified: end-to-end
relative error ~3.7e-3, at the fp32 reordering noise floor of this
amplifying iteration).
"""

import os

import numpy as np
import ml_dtypes

RANKS = 8
P = 128
NREAL = 10000
NK = 10240            # padded global row count (80 k-tiles)
ML = 1280             # local rows per core (10 m-tiles)
KT = NK // P          # 80
MT = ML // P          # 10
F = 512
FT = F // P           # 4
HID = 256
C = 16
NPOST = 4
NRES = 50             # raw-adj k-tiles kept resident in SBUF after 1st pass
NRESN = 32            # normed-adj k-tiles cached in SBUF between ph2 and ph4
NCH = 3               # free-dim chunks of ML: 512/512/256
CHUNKS = [(0, 512), (512, 1024), (1024, 1280)]

PHASES = int(os.environ.get("CPGNN_PHASES", "5"))

_CACHE = {}


def _mix_order(n_cache, n_total):
    """Interleave cached (0..n_cache-1) and streamed (n_cache..) k indices so
    DMA of streamed tiles overlaps PE work on cached tiles evenly."""
    cached = list(range(n_cache))
    streamed = list(range(n_cache, n_total))
    order = []
    ic = si = 0
    for i in range(n_total):
        want_stream = streamed and (si + 1) / len(streamed) <= (i + 1) / n_total
        if si < len(streamed) and (ic >= len(cached) or want_stream):
            order.append(streamed[si]); si += 1
        else:
            order.append(cached[ic]); ic += 1
    assert sorted(order) == list(range(n_total))
    return order


def _build_and_compile():
    import concourse.mybir as mybir
    import concourse.tile as tile
    from concourse import bacc
    from concourse.masks import make_identity

    dt = mybir.dt
    f32 = dt.float32
    bf16 = dt.bfloat16
    AF = mybir.ActivationFunctionType

    nc = bacc.Bacc("TRN2", target_bir_lowering=False, debug=False,
                   num_devices=RANKS)

    adjTn = nc.dram_tensor("adjTn", [NK, ML], bf16, kind="ExternalInput").ap()
    adjTr = nc.dram_tensor("adjTr", [NK, ML], bf16, kind="ExternalInput").ap()
    xT = nc.dram_tensor("xT", [F, NK], bf16, kind="ExternalInput").ap()
    w1 = nc.dram_tensor("w1", [F, HID], bf16, kind="ExternalInput").ap()
    w2 = nc.dram_tensor("w2", [HID, C], bf16, kind="ExternalInput").ap()
    hm = nc.dram_tensor("hm", [C, C], bf16, kind="ExternalInput").ap()
    b1 = nc.dram_tensor("b1", [HID, 1], f32, kind="ExternalInput").ap()
    b2c = nc.dram_tensor("b2c", [C, 1], f32, kind="ExternalInput").ap()
    outT = nc.dram_tensor("outT", [C, ML], f32, kind="ExternalOutput").ap()

    rg = [list(range(RANKS))]

    with tile.TileContext(nc) as tc:
        with tc.tile_pool(name="const", bufs=1) as const_pool, \
             tc.tile_pool(name="persist", bufs=1) as persist, \
             tc.tile_pool(name="dram", bufs=1, space="DRAM") as dram_pool:

            # ---- constants ----
            w1_sb = const_pool.tile([P, FT, HID], bf16)
            nc.sync.dma_start(w1_sb[:], w1.rearrange("(kt p) h -> p kt h", p=P))
            w2_sb = const_pool.tile([P, 2, C], bf16)
            nc.sync.dma_start(w2_sb[:], w2.rearrange("(kt p) c -> p kt c", p=P))
            h_sb = const_pool.tile([C, C], bf16)
            nc.sync.dma_start(h_sb[:], hm[:])
            b1_sb = const_pool.tile([P, 2, 1], f32)
            nc.sync.dma_start(b1_sb[:], b1.rearrange("(t p) o -> p t o", p=P))
            b2c_sb = const_pool.tile([C, 1], f32)
            nc.sync.dma_start(b2c_sb[:], b2c[:])
            ones16_sb = const_pool.tile([C, 1], f32)
            nc.gpsimd.memset(ones16_sb[:], 1.0)
            ones1_sb = const_pool.tile([1, C], f32)
            nc.gpsimd.memset(ones1_sb[:], 1.0)

            # ---- persistent intermediates ----
            h1t_sb = persist.tile([P, 2, ML], bf16)        # h.T  [HID, ML]
            hw2f_sb = persist.tile([P, KT, C], bf16)       # gathered h@W2 [NK, C]
            et_sb = persist.tile([C, ML], f32)             # E_hat.T local
            btcat_sb = persist.tile([C, NK], bf16)         # gathered B.T
            y_sb = persist.tile([P, KT, C], bf16)          # (B @ H) K-major

            # ================= phase 1: XW1 = X @ W1  [NK, HID] =============
            if PHASES >= 1:
              with tc.tile_pool(name="xw1p", bufs=1) as xw1p:
                xw1_sb = xw1p.tile([P, KT, HID], bf16)
                with tc.tile_pool(name="ph1", bufs=1) as ph1, \
                     tc.tile_pool(name="ps1", bufs=4, space="PSUM") as ps1:
                    xT_sb = ph1.tile([P, FT, NK], bf16)
                    xT_r = xT.rearrange("(kt p) n -> p kt n", p=P)
                    XCH = 8
                    xw = NK // XCH
                    for c in range(XCH):
                        nc.sync.dma_start(xT_sb[:, :, c * xw:(c + 1) * xw],
                                          xT_r[:, :, c * xw:(c + 1) * xw])
                    for m in range(KT):
                        psum1 = ps1.tile([P, HID], f32, name="psum1")
                        for kf in range(FT):
                            nc.tensor.matmul(
                                psum1[:],
                                xT_sb[:, kf, m * P:(m + 1) * P],
                                w1_sb[:, kf, :],
                                start=(kf == 0), stop=(kf == FT - 1))
                        nc.scalar.activation(xw1_sb[:, m, :], psum1[:], AF.Copy)

                # ============= phase 2: H1T = relu(XW1.T @ adjTn + b1) ======
                if PHASES >= 2:
                    # cachen outlives phase 2 (reused in phase 4)
                    cachen_cm = tc.tile_pool(name="cachen", bufs=1)
                    cachen = cachen_cm.__enter__()
                    adjn_res = cachen.tile([P, NRESN, ML], bf16)
                    with tc.tile_pool(name="ph2s", bufs=4) as ph2s, \
                         tc.tile_pool(name="ps2", bufs=1, space="PSUM") as ps2:
                        psum_h0 = ps2.tile([P, ML], f32, name="psum_h0")
                        psum_h1 = ps2.tile([P, ML], f32, name="psum_h1")
                        psum_h = [psum_h0, psum_h1]
                        for k in range(KT):
                            if k < NRESN:
                                nc.sync.dma_start(adjn_res[:, k, :],
                                                  adjTn[k * P:(k + 1) * P, :])
                                src = adjn_res[:, k, :]
                            else:
                                adjn_k = ph2s.tile([P, ML], bf16, name="adjn_k")
                                nc.sync.dma_start(adjn_k[:],
                                                  adjTn[k * P:(k + 1) * P, :])
                                src = adjn_k[:]
                            for mh in range(2):
                                for (n0, n1) in CHUNKS:
                                    nc.tensor.matmul(
                                        psum_h[mh][:, n0:n1],
                                        xw1_sb[:, k, mh * P:(mh + 1) * P],
                                        src[:, n0:n1],
                                        start=(k == 0), stop=(k == KT - 1))
                        for mh in range(2):
                            nc.scalar.activation(h1t_sb[:, mh, :], psum_h[mh][:],
                                                 AF.Relu, bias=b1_sb[:, mh, :])

                # ================= phase 3: hW2 = h @ W2  [ML, C], all-gather ===
                if PHASES >= 3:
                    with tc.tile_pool(name="ph3", bufs=1) as ph3, \
                         tc.tile_pool(name="ps3", bufs=4, space="PSUM") as ps3:
                        hw2_sb = ph3.tile([P, MT, C], bf16)
                        for m in range(MT):
                            psum3 = ps3.tile([P, C], f32, name="psum3")
                            for kh in range(2):
                                nc.tensor.matmul(
                                    psum3[:],
                                    h1t_sb[:, kh, m * P:(m + 1) * P],
                                    w2_sb[:, kh, :],
                                    start=(kh == 0), stop=(kh == 1))
                            nc.scalar.activation(hw2_sb[:, m, :], psum3[:], AF.Copy)
                        hw2loc_dram = dram_pool.tile([ML, C], bf16)
                        nc.sync.dma_start(
                            hw2loc_dram.rearrange("(mt p) c -> p mt c", p=P),
                            hw2_sb[:])
                        hw2full_dram = dram_pool.tile([NK, C], bf16,
                                                      addr_space="Shared")
                        nc.gpsimd.collective_compute(
                            "AllGather", mybir.AluOpType.bypass, replica_groups=rg,
                            ins=[hw2loc_dram[:].opt()], outs=[hw2full_dram[:].opt()])
                        nc.sync.dma_start(
                            hw2f_sb[:],
                            hw2full_dram.rearrange("(kt p) c -> p kt c", p=P))

                # ====== phase 4: logitsT = hW2_full.T @ adjTn; softmax; E_hat ===
                if PHASES >= 4:
                    with tc.tile_pool(name="ph4s", bufs=4) as ph4s, \
                         tc.tile_pool(name="ph4", bufs=1) as ph4, \
                         tc.tile_pool(name="ps4", bufs=1, space="PSUM") as ps4:
                        psum_l0 = ps4.tile([P, ML], f32, name="psum_l0",
                                           tag="ph4big")
                        psum_l1 = ps4.tile([P, ML], f32, name="psum_l1")
                        psum_ls = [psum_l0, psum_l1]
                        korder = _mix_order(NRESN, KT)
                        for ki, k in enumerate(korder):
                            j = ki % 2  # PE column strip
                            if k < NRESN:
                                src = adjn_res[:, k, :]
                            else:
                                adjn_k2 = ph4s.tile([P, ML], bf16, name="adjn_k2")
                                nc.sync.dma_start(adjn_k2[:],
                                                  adjTn[k * P:(k + 1) * P, :])
                                src = adjn_k2[:]
                            for (n0, n1) in CHUNKS:
                                nc.tensor.matmul(
                                    psum_ls[j][32 * j:32 * j + C, n0:n1],
                                    hw2f_sb[:, k, :],
                                    src[:, n0:n1],
                                    start=(ki < 2), stop=(ki >= KT - 2),
                                    tile_position=(0, 32 * j))
                        # sum the 2 column-strip partials -> logitsT [C, ML]
                        # (DVE may read only one PSUM operand: stage strip 1 via SBUF)
                        lt_s1 = ph4.tile([C, ML], f32, name="lt_s1")
                        nc.scalar.activation(lt_s1[:], psum_l1[32:32 + C, :], AF.Copy)
                        lt_sum = ph4.tile([C, ML], f32, name="lt_sum")
                        nc.vector.tensor_add(lt_sum[:], psum_l0[0:C, :], lt_s1[:])
                        # transposed softmax over classes (partition dim):
                        # expT = exp(logitsT + b2); sums = 1^T expT (PE);
                        # bcast sums over partitions (PE); E = expT/sums - 1/C
                        expT_sb = ph4.tile([C, ML], f32)
                        nc.scalar.activation(expT_sb[:], lt_sum[:], AF.Exp,
                                             bias=b2c_sb[:])
                        sums_ps = ps4.tile([1, ML], f32, name="sums_ps",
                                           tag="ph4big")
                        for (n0, n1) in CHUNKS:
                            nc.tensor.matmul(sums_ps[:, n0:n1], ones16_sb[:],
                                             expT_sb[:, n0:n1],
                                             start=True, stop=True)
                        sumsr_sb = ph4.tile([1, ML], f32)
                        nc.scalar.activation(sumsr_sb[:], sums_ps[:], AF.Copy)
                        bc_ps = ps4.tile([C, ML], f32, name="bc_ps", tag="ph4big")
                        for (n0, n1) in CHUNKS:
                            nc.tensor.matmul(bc_ps[:, n0:n1], ones1_sb[:],
                                             sumsr_sb[:, n0:n1],
                                             start=True, stop=True)
                        rcp_sb = ph4.tile([C, ML], f32)
                        nc.vector.reciprocal(rcp_sb[:], bc_ps[:])
                        et_pre = ph4.tile([C, ML], f32)
                        nc.vector.tensor_mul(et_pre[:], expT_sb[:], rcp_sb[:])
                        nc.vector.tensor_scalar_add(et_sb[:], et_pre[:], -1.0 / C)
                        etb_sb = ph4.tile([C, ML], bf16)
                        nc.scalar.activation(etb_sb[:], et_sb[:], AF.Copy)

                        # all-gather E_hat.T blocks -> btcat
                        et_dram = dram_pool.tile([C, ML], bf16)
                        nc.sync.dma_start(et_dram[:], etb_sb[:])
                        btfull0 = dram_pool.tile([P, ML], bf16, addr_space="Shared")
                        nc.gpsimd.collective_compute(
                            "AllGather", mybir.AluOpType.bypass, replica_groups=rg,
                            ins=[et_dram[:].opt()], outs=[btfull0[:].opt()])
                        nc.sync.dma_start(
                            btcat_sb.rearrange("c (r m) -> c r m", r=RANKS),
                            btfull0.rearrange("(r c) m -> c r m", c=C))

                if PHASES >= 2:
                    cachen_cm.__exit__(None, None, None)

            # ================= phase 5: post-process iterations =============
            if PHASES >= 5:
                with tc.tile_pool(name="res", bufs=1) as res_pool, \
                     tc.tile_pool(name="ph5s", bufs=4) as ph5s, \
                     tc.tile_pool(name="ph5", bufs=1) as ph5, \
                     tc.tile_pool(name="ps5y", bufs=2, space="PSUM") as ps5y, \
                     tc.tile_pool(name="ps5b", bufs=1, space="PSUM") as ps5b:
                    adjr_res = res_pool.tile([P, NRES, ML], bf16)
                    for it in range(NPOST):
                        # Y = B @ H in K-major layout, from gathered B.T blocks
                        YB = 32
                        for mb in range(0, KT, YB):
                            nb = min(YB, KT - mb)
                            psum_y = ps5y.tile([P, YB, C], f32, name="psum_y")
                            for j in range(nb):
                                m = mb + j
                                nc.tensor.matmul(psum_y[:, j, :],
                                                 btcat_sb[:, m * P:(m + 1) * P],
                                                 h_sb[:], start=True, stop=True)
                            nc.scalar.activation(y_sb[:, mb:mb + nb, :],
                                                 psum_y[:, :nb, :], AF.Copy)
                        # T.T = Y.T @ adjTr  (accumulate over k-tiles)
                        psum_b0 = ps5b.tile([P, ML], f32, name="psum_b0")
                        psum_b1 = ps5b.tile([P, ML], f32, name="psum_b1")
                        psum_bs = [psum_b0, psum_b1]
                        korder5 = _mix_order(NRES, KT) if it > 0 else list(range(KT))
                        for ki, k in enumerate(korder5):
                            j = ki % 2  # PE column strip
                            if k < NRES:
                                if it == 0:
                                    nc.sync.dma_start(
                                        adjr_res[:, k, :],
                                        adjTr[k * P:(k + 1) * P, :])
                                src = adjr_res[:, k, :]
                            else:
                                adjr_k = ph5s.tile([P, ML], bf16, name="adjr_k")
                                nc.sync.dma_start(adjr_k[:],
                                                  adjTr[k * P:(k + 1) * P, :])
                                src = adjr_k[:]
                            for (n0, n1) in CHUNKS:
                                nc.tensor.matmul(
                                    psum_bs[j][32 * j:32 * j + C, n0:n1],
                                    y_sb[:, k, :],
                                    src[:, n0:n1],
                                    start=(ki < 2), stop=(ki >= KT - 2),
                                    tile_position=(0, 32 * j))
                        bt_s1 = ph5.tile([C, ML], f32, name="bt_s1", bufs=2)
                        nc.scalar.activation(bt_s1[:], psum_b1[32:32 + C, :], AF.Copy)
                        btsum = ph5.tile([C, ML], f32, name="btsum", bufs=2)
                        nc.vector.tensor_add(btsum[:], psum_b0[0:C, :], bt_s1[:])
                        if it < NPOST - 1:
                            btnb = ph5.tile([C, ML], bf16, name="btnb", bufs=2)
                            nc.vector.tensor_add(btnb[:], btsum[:], et_sb[:])
                            bt_dram = dram_pool.tile([C, ML], bf16,
                                                     name=f"bt_dram{it}")
                            nc.sync.dma_start(bt_dram[:], btnb[:])
                            btfull = dram_pool.tile([P, ML], bf16,
                                                    name=f"btfull{it}",
                                                    addr_space="Shared")
                            nc.gpsimd.collective_compute(
                                "AllGather", mybir.AluOpType.bypass,
                                replica_groups=rg,
                                ins=[bt_dram[:].opt()], outs=[btfull[:].opt()])
                            nc.sync.dma_start(
                                btcat_sb.rearrange("c (r m) -> c r m", r=RANKS),
                                btfull.rearrange("(r c) m -> c r m", c=C))
                        else:
                            btn = ph5.tile([C, ML], f32, name="btn")
                            nc.vector.tensor_add(btn[:], btsum[:], et_sb[:])
                            outT_sb = ph5.tile([C, ML], f32, name="outT_sb")
                            nc.vector.tensor_scalar_add(outT_sb[:], btn[:],
                                                        1.0 / C)
                            nc.sync.dma_start(outT[:], outT_sb[:])
            else:
                # truncated build: still write the output tensor
                with tc.tile_pool(name="dummy", bufs=1) as dummy:
                    dpad = dummy.tile([C, ML], f32)
                    nc.gpsimd.memset(dpad[:], 0.0)
                    nc.sync.dma_start(outT[:], dpad[:])

    nc.compile()
    return nc


def _get_compiled():
    if "nc" not in _CACHE:
        _CACHE["nc"] = _build_and_compile()
    return _CACHE["nc"]


def _prep_inputs(raw_adj, normed_adj, features, W1, b1, W2, b2, H):
    bf = ml_dtypes.bfloat16
    xTp = np.zeros((F, NK), dtype=bf)
    xTp[:, :NREAL] = np.ascontiguousarray(features.T).astype(bf)
    w1b = np.ascontiguousarray(W1).astype(bf)
    w2b = np.ascontiguousarray(W2).astype(bf)
    hb = np.ascontiguousarray(H).astype(bf)
    b1c = np.asarray(b1, dtype=np.float32).reshape(HID, 1).copy()
    b2col = np.asarray(b2, dtype=np.float32).reshape(C, 1).copy()
    in_maps = []
    for r in range(RANKS):
        r0 = r * ML
        r1 = min(r0 + ML, NREAL)
        nr = r1 - r0
        an = np.zeros((NK, ML), dtype=bf)
        an[:NREAL, :nr] = np.ascontiguousarray(normed_adj[r0:r1].T).astype(bf)
        ar = np.zeros((NK, ML), dtype=bf)
        ar[:NREAL, :nr] = np.ascontiguousarray(raw_adj[r0:r1].T).astype(bf)
        in_maps.append({
            "adjTn": an, "adjTr": ar, "xT": xTp, "w1": w1b, "w2": w2b,
            "hm": hb, "b1": b1c, "b2c": b2col,
        })
    return in_maps


def run_on_device(in_maps, trace=False):
    from concourse import bass_utils
    nc = _get_compiled()
    return bass_utils.run_bass_kernel_spmd(
        nc, in_maps, core_ids=list(range(RANKS)), trace=trace)


def kernel(raw_adj, normed_adj, features, y_onehot, train_mask,
           W1, b1, W2, b2, H):
    in_maps = _prep_inputs(np.asarray(raw_adj), np.asarray(normed_adj),
                           np.asarray(features), np.asarray(W1),
                           np.asarray(b1), np.asarray(W2), np.asarray(b2),
                           np.asarray(H))
    res = run_on_device(in_maps)
    parts = []
    for r in range(RANKS):
        o = np.asarray(res.results[r]["outT"], dtype=np.float32)  # [C, ML]
        parts.append(o.T)
    full = np.concatenate(parts, axis=0)[:NREAL]
    return np.ascontiguousarray(full).astype(np.float32)



# revision 11
# speedup vs baseline: 1.1723x; 1.1723x over previous
"""CPGNN (compatibility-guided GNN) kernel for 8 Trainium2 NeuronCores.

Reference computation (N=10000, F=512, HID=256, C=16, 4 post iterations):
    h      = relu(normed_adj @ (features @ W1) + b1)
    logits = normed_adj @ (h @ W2) + b2
    E_hat  = softmax(logits) - 1/C
    B_hat  = E_hat;  4x: B_hat = E_hat + raw_adj @ (B_hat @ H)
    out    = B_hat + 1/C

Sharding: adjacency rows over 8 cores (1280 rows/core, tail padded),
adjacency shards uploaded TRANSPOSED (K-major [10240, 1280]) in
**fp8 e3m4** (normed_adj scaled by 2^15, raw_adj by 2^10; descale is
folded into the activation `scale` at PSUM-drain time).  fp8 halves
the dominant DMA traffic and lets the whole raw-adj shard stay
SBUF-resident across all 4 post iterations.  Numpy simulation of this
exact quantization chain gives rel-err 3.9e-3 (gate: 2e-2).

Other structure:
 - phase 1 computes only this rank's 1280 rows of X@W1 and all-gathers
   the [10240, 256] bf16 result (saves ~30us of replicated PE work and
   9MB of xT DMA per core).
 - the M=16 adj matmuls (logits, post-iterations) are packed 4-wide
   into PE column groups via tile_position; the K=16 Y=B@H matmuls are
   packed 4-wide into PE row groups (B.T gathered into 4 partition-
   offset copies).
 - DMA issue order is orchestrated so the raw-adj prefetch and the
   normed-adj stream fill the gaps behind collectives.
"""

import os

import numpy as np
import ml_dtypes

RANKS = 8
P = 128
NREAL = 10000
NK = 10240            # padded global row count (80 k-tiles)
ML = 1280             # local rows per core (10 m-tiles)
KT = NK // P          # 80
MT = ML // P          # 10
F = 512
FT = F // P           # 4
HID = 256
C = 16
NPOST = 4
SN = float(2 ** 15)   # normed_adj fp8 scale
SR = float(2 ** 10)   # raw_adj fp8 scale
CN = 28               # normed-adj k-tiles cached in SBUF for phase 4
NCH = 3
CHUNKS = [(0, 512), (512, 1024), (1024, 1280)]
NSTRIP = 4            # PE column-group packing for M=16 matmuls

PHASES = int(os.environ.get("CPGNN_PHASES", "5"))
YPACK = int(os.environ.get("CPGNN_YPACK", "4"))     # row groups for Y=B@H
BSTRIP = int(os.environ.get("CPGNN_BSTRIP", str(NSTRIP)))  # col strips ph5

_CACHE = {}


def _mix_order(n_cache, n_total):
    """Interleave cached (0..n_cache-1) and streamed (n_cache..) k indices so
    DMA of streamed tiles overlaps PE work on cached tiles evenly."""
    cached = list(range(n_cache))
    streamed = list(range(n_cache, n_total))
    order = []
    ic = si = 0
    for i in range(n_total):
        want_stream = streamed and (si + 1) / len(streamed) <= (i + 1) / n_total
        if si < len(streamed) and (ic >= len(cached) or want_stream):
            order.append(streamed[si]); si += 1
        else:
            order.append(cached[ic]); ic += 1
    assert sorted(order) == list(range(n_total))
    return order


def _build_and_compile():
    import concourse.mybir as mybir
    import concourse.tile as tile
    from concourse import bacc

    dt = mybir.dt
    f32 = dt.float32
    bf16 = dt.bfloat16
    f8 = dt.float8e3
    AF = mybir.ActivationFunctionType

    nc = bacc.Bacc("TRN2", target_bir_lowering=False, debug=False,
                   num_devices=RANKS)

    adjTn = nc.dram_tensor("adjTn", [NK, ML], f8, kind="ExternalInput").ap()
    adjTr = nc.dram_tensor("adjTr", [NK, ML], f8, kind="ExternalInput").ap()
    xTl = nc.dram_tensor("xTl", [F, ML], bf16, kind="ExternalInput").ap()
    w1 = nc.dram_tensor("w1", [F, HID], bf16, kind="ExternalInput").ap()
    w2 = nc.dram_tensor("w2", [HID, C], bf16, kind="ExternalInput").ap()
    hm = nc.dram_tensor("hm", [C, C], bf16, kind="ExternalInput").ap()
    b1 = nc.dram_tensor("b1", [HID, 1], f32, kind="ExternalInput").ap()
    b2c = nc.dram_tensor("b2c", [C, 1], f32, kind="ExternalInput").ap()
    outT = nc.dram_tensor("outT", [C, ML], f32, kind="ExternalOutput").ap()

    rg = [list(range(RANKS))]

    with tile.TileContext(nc) as tc:
        with tc.tile_pool(name="const", bufs=1) as const_pool, \
             tc.tile_pool(name="persist", bufs=1) as persist, \
             tc.tile_pool(name="dram", bufs=1, space="DRAM") as dram_pool:

            # ---- constants ----
            w1_sb = const_pool.tile([P, FT, HID], bf16)
            nc.sync.dma_start(w1_sb[:], w1.rearrange("(kt p) h -> p kt h", p=P))
            w2_sb = const_pool.tile([P, 2, C], bf16)
            nc.sync.dma_start(w2_sb[:], w2.rearrange("(kt p) c -> p kt c", p=P))
            h_sb = const_pool.tile([C, C], bf16)
            nc.sync.dma_start(h_sb[:], hm[:])
            b1_sb = const_pool.tile([P, 2, 1], f32)
            nc.sync.dma_start(b1_sb[:], b1.rearrange("(t p) o -> p t o", p=P))
            b2c_sb = const_pool.tile([C, 1], f32)
            nc.sync.dma_start(b2c_sb[:], b2c[:])
            ones16_sb = const_pool.tile([C, 1], f32)
            nc.gpsimd.memset(ones16_sb[:], 1.0)
            ones1_sb = const_pool.tile([1, C], f32)
            nc.gpsimd.memset(ones1_sb[:], 1.0)

            # ---- persistent intermediates ----
            h1t_sb = persist.tile([P, 2, ML], bf16)        # h.T  [HID, ML]
            hw2f_sb = persist.tile([P, KT, C], bf16)       # gathered h@W2 [NK, C]
            y_sb = persist.tile([P, KT, C], bf16)          # (B @ H) K-major
            et_sb = persist.tile([C, ML], f32)             # E_hat.T local

            # raw-adj shard, fully SBUF-resident in fp8 (100 KiB/partition)
            adjr_cm = tc.tile_pool(name="adjr", bufs=1)
            adjr = adjr_cm.__enter__()
            adjr_res = adjr.tile([P, KT, ML], f8)
            radjr = [0]      # prefetch progress

            def prefetch_adjr(n):
                k0 = radjr[0]
                for k in range(k0, min(k0 + n, KT)):
                    nc.sync.dma_start(adjr_res[:, k, :],
                                      adjTr[k * P:(k + 1) * P, :])
                    radjr[0] = k + 1

            # normed-adj cache for phase 4
            cachen_cm = tc.tile_pool(name="cachen", bufs=1)
            cachen = cachen_cm.__enter__()
            adjn_res = cachen.tile([P, CN, ML], f8)

            # =========== phase 1: local strip of X@W1, then all-gather ======
            xw1p_cm = tc.tile_pool(name="xw1p", bufs=1, side="right")
            xw1p = xw1p_cm.__enter__()
            xw1_sb = xw1p.tile([P, KT, HID], bf16)
            if PHASES >= 1:
                with tc.tile_pool(name="ph1", bufs=1, side="right") as ph1, \
                     tc.tile_pool(name="ps1", bufs=4, space="PSUM") as ps1:
                    xt_sb = ph1.tile([P, FT, ML], bf16)
                    nc.sync.dma_start(xt_sb[:],
                                      xTl.rearrange("(kt p) n -> p kt n", p=P))
                    # dep-free prefetches fill the queue behind the gather
                    for k in range(CN):
                        nc.sync.dma_start(adjn_res[:, k, :],
                                          adjTn[k * P:(k + 1) * P, :])
                    prefetch_adjr(12)
                    xw1l_sb = ph1.tile([P, MT, HID], bf16)
                    for m in range(MT):
                        psum1 = ps1.tile([P, HID], f32, name="psum1")
                        for kf in range(FT):
                            nc.tensor.matmul(
                                psum1[:],
                                xt_sb[:, kf, m * P:(m + 1) * P],
                                w1_sb[:, kf, :],
                                start=(kf == 0), stop=(kf == FT - 1))
                        nc.scalar.activation(xw1l_sb[:, m, :], psum1[:], AF.Copy)
                    xw1loc_dram = dram_pool.tile([ML, HID], bf16)
                    nc.sync.dma_start(
                        xw1loc_dram.rearrange("(mt p) h -> p mt h", p=P),
                        xw1l_sb[:])
                    xw1full = dram_pool.tile([NK, HID], bf16,
                                             addr_space="Shared")
                    nc.gpsimd.collective_compute(
                        "AllGather", mybir.AluOpType.bypass, replica_groups=rg,
                        ins=[xw1loc_dram[:].opt()], outs=[xw1full[:].opt()])
                    xw1full_r = xw1full.rearrange("(kt p) h -> p kt h", p=P)
                    for cc in range(8):
                        nc.sync.dma_start(xw1_sb[:, cc * 10:cc * 10 + 10, :],
                                          xw1full_r[:, cc * 10:cc * 10 + 10, :])

            # =========== phase 2: H1T = relu(XW1.T @ adjTn / SN + b1) =======
            if PHASES >= 2:
                with tc.tile_pool(name="ph2s", bufs=6, side="right") as ph2s, \
                     tc.tile_pool(name="ps2", bufs=1, space="PSUM") as ps2:
                    psum_h0 = ps2.tile([P, ML], f32, name="psum_h0")
                    psum_h1 = ps2.tile([P, ML], f32, name="psum_h1")
                    psum_h = [psum_h0, psum_h1]
                    for k in range(KT):
                        if k < CN:
                            src = adjn_res[:, k, :]
                        else:
                            adjn_k = ph2s.tile([P, ML], f8, name="adjn_k")
                            nc.sync.dma_start(adjn_k[:],
                                              adjTn[k * P:(k + 1) * P, :])
                            src = adjn_k[:]
                        if radjr[0] < 60:
                            prefetch_adjr(1)
                        for mh in range(2):
                            for (n0, n1) in CHUNKS:
                                nc.tensor.matmul(
                                    psum_h[mh][:, n0:n1],
                                    xw1_sb[:, k, mh * P:(mh + 1) * P],
                                    src[:, n0:n1],
                                    start=(k == 0), stop=(k == KT - 1))
                    for mh in range(2):
                        nc.scalar.activation(h1t_sb[:, mh, :], psum_h[mh][:],
                                             AF.Relu, bias=b1_sb[:, mh, :],
                                             scale=1.0 / SN)
            xw1p_cm.__exit__(None, None, None)

            # =========== phase 3: hW2 = h @ W2  [ML, C], all-gather =========
            if PHASES >= 3:
                with tc.tile_pool(name="ph3", bufs=1, side="right") as ph3, \
                     tc.tile_pool(name="ps3", bufs=4, space="PSUM") as ps3:
                    hw2_sb = ph3.tile([P, MT, C], bf16)
                    for m in range(MT):
                        psum3 = ps3.tile([P, C], f32, name="psum3")
                        for kh in range(2):
                            nc.tensor.matmul(
                                psum3[:],
                                h1t_sb[:, kh, m * P:(m + 1) * P],
                                w2_sb[:, kh, :],
                                start=(kh == 0), stop=(kh == 1))
                        nc.scalar.activation(hw2_sb[:, m, :], psum3[:], AF.Copy)
                    hw2loc_dram = dram_pool.tile([ML, C], bf16)
                    nc.sync.dma_start(
                        hw2loc_dram.rearrange("(mt p) c -> p mt c", p=P),
                        hw2_sb[:])
                    prefetch_adjr(10)
                    hw2full_dram = dram_pool.tile([NK, C], bf16,
                                                  addr_space="Shared")
                    nc.gpsimd.collective_compute(
                        "AllGather", mybir.AluOpType.bypass, replica_groups=rg,
                        ins=[hw2loc_dram[:].opt()], outs=[hw2full_dram[:].opt()])
                    nc.sync.dma_start(
                        hw2f_sb[:],
                        hw2full_dram.rearrange("(kt p) c -> p kt c", p=P))

            # ====== phase 4: logitsT = hW2f.T @ adjTn; softmax; E_hat =======
            yfullKM0 = dram_pool.tile([NK, C], bf16, name="yfullKM0",
                                      addr_space="Shared")
            if PHASES >= 4:
                with tc.tile_pool(name="ph4s", bufs=6, side="right") as ph4s, \
                     tc.tile_pool(name="ph4", bufs=1, side="right") as ph4, \
                     tc.tile_pool(name="ps4", bufs=1, space="PSUM") as ps4:
                    psum_l = ps4.tile([P, ML], f32, name="psum_l", tag="ph4big")
                    korder = _mix_order(CN, KT)
                    for ki, k in enumerate(korder):
                        j = ki % NSTRIP
                        if k < CN:
                            src = adjn_res[:, k, :]
                        else:
                            adjn_k2 = ph4s.tile([P, ML], f8, name="adjn_k2")
                            nc.sync.dma_start(adjn_k2[:],
                                              adjTn[k * P:(k + 1) * P, :])
                            src = adjn_k2[:]
                        if ki % 8 == 0:
                            prefetch_adjr(1)
                        for (n0, n1) in CHUNKS:
                            nc.tensor.matmul(
                                psum_l[32 * j:32 * j + C, n0:n1],
                                hw2f_sb[:, k, :],
                                src[:, n0:n1],
                                start=(ki < NSTRIP), stop=(ki >= KT - NSTRIP),
                                tile_position=(0, 32 * j),
                                skip_group_check=True)
                    prefetch_adjr(KT)  # any remainder
                    # reduce the 4 column strips (stage strips via ACT; DVE
                    # may read at most one PSUM operand per op)
                    s0 = ph4.tile([C, ML], f32, name="sA")
                    nc.scalar.activation(s0[:], psum_l[0:C, :], AF.Copy)
                    s1 = ph4.tile([C, ML], f32, name="sB")
                    nc.scalar.activation(s1[:], psum_l[32:32 + C, :], AF.Copy)
                    s2 = ph4.tile([C, ML], f32, name="sC")
                    nc.scalar.activation(s2[:], psum_l[64:64 + C, :], AF.Copy)
                    s3 = ph4.tile([C, ML], f32, name="sD")
                    nc.scalar.activation(s3[:], psum_l[96:96 + C, :], AF.Copy)
                    u0 = ph4.tile([C, ML], f32, name="sE")
                    nc.vector.tensor_add(u0[:], s0[:], s1[:])
                    u1 = ph4.tile([C, ML], f32, name="sA")
                    nc.vector.tensor_add(u1[:], s2[:], s3[:])
                    lt = ph4.tile([C, ML], f32, name="sB")
                    nc.vector.tensor_add(lt[:], u0[:], u1[:])
                    # transposed softmax: expT = exp(lt/SN + b2);
                    # sums = 1^T expT (PE); bcast over partitions (PE)
                    expT = ph4.tile([C, ML], f32, name="sC")
                    nc.scalar.activation(expT[:], lt[:], AF.Exp,
                                         bias=b2c_sb[:], scale=1.0 / SN)
                    sums_ps = ps4.tile([1, ML], f32, name="sums_ps",
                                       tag="ph4big")
                    for (n0, n1) in CHUNKS:
                        nc.tensor.matmul(sums_ps[:, n0:n1], ones16_sb[:],
                                         expT[:, n0:n1],
                                         start=True, stop=True)
                    sumsr = ph4.tile([1, ML], f32, name="sD")
                    nc.scalar.activation(sumsr[:], sums_ps[:], AF.Copy)
                    bc_ps = ps4.tile([C, ML], f32, name="bc_ps", tag="ph4big")
                    for (n0, n1) in CHUNKS:
                        nc.tensor.matmul(bc_ps[:, n0:n1], ones1_sb[:],
                                         sumsr[:, n0:n1],
                                         start=True, stop=True)
                    rcp = ph4.tile([C, ML], f32, name="sA")
                    nc.vector.reciprocal(rcp[:], bc_ps[:])
                    etp = ph4.tile([C, ML], f32, name="sB")
                    nc.vector.tensor_mul(etp[:], expT[:], rcp[:])
                    nc.vector.tensor_scalar_add(et_sb[:], etp[:], -1.0 / C)
                    etb = ph4.tile([C, ML], bf16, name="etb")
                    nc.scalar.activation(etb[:], et_sb[:], AF.Copy)
                    # y0 = E @ H for the local block, gathered node-major so
                    # it lands directly in the phase-5 lhsT layout
                    psum_ym0 = ps4.tile([P, MT, C], f32, name="psum_ym0")
                    for m in range(MT):
                        nc.tensor.matmul(psum_ym0[:, m, :],
                                         etb[:, m * P:(m + 1) * P], h_sb[:],
                                         start=True, stop=True)
                    yloc0 = ph4.tile([P, MT, C], bf16, name="yloc0")
                    nc.scalar.activation(yloc0[:], psum_ym0[:], AF.Copy)
                    yloc0_dram = dram_pool.tile([ML, C], bf16, name="yloc0d")
                    nc.sync.dma_start(
                        yloc0_dram.rearrange("(mt p) c -> p mt c", p=P),
                        yloc0[:])
                    nc.gpsimd.collective_compute(
                        "AllGather", mybir.AluOpType.bypass, replica_groups=rg,
                        ins=[yloc0_dram[:].opt()], outs=[yfullKM0[:].opt()])
                    nc.sync.dma_start(
                        y_sb[:],
                        yfullKM0.rearrange("(kt p) c -> p kt c", p=P))
            cachen_cm.__exit__(None, None, None)

            # =========== phase 5: post-process iterations ===================
            # Iterate on y = B@H: each rank computes only its LOCAL y block
            # (B = E + usum computed locally) and all-gathers y node-major,
            # which is exactly the lhsT layout the big matmul needs.
            if PHASES >= 5:
                with tc.tile_pool(name="bt", bufs=1) as bt, \
                     tc.tile_pool(name="ps5m", bufs=1, space="PSUM") as ps5m, \
                     tc.tile_pool(name="ps5b", bufs=1, space="PSUM") as ps5b:
                    for it in range(NPOST):
                        # T.T = Y.T @ adjTr (all k-tiles SBUF-resident)
                        psum_b = ps5b.tile([P, ML], f32, name="psum_b")
                        for ki in range(KT):
                            j = ki % BSTRIP
                            for (n0, n1) in CHUNKS:
                                nc.tensor.matmul(
                                    psum_b[32 * j:32 * j + C, n0:n1],
                                    y_sb[:, ki, :],
                                    adjr_res[:, ki, n0:n1],
                                    start=(ki < BSTRIP),
                                    stop=(ki >= KT - BSTRIP),
                                    tile_position=(0, 32 * j),
                                    skip_group_check=True)
                        stage = []
                        for j in range(BSTRIP):
                            tj = bt.tile([C, ML], f32, name=f"p{j}")
                            nc.scalar.activation(
                                tj[:], psum_b[32 * j:32 * j + C, :],
                                AF.Copy, scale=1.0 / SR)
                            stage.append(tj)
                        gen = 0
                        while len(stage) > 1:
                            nxt = []
                            for i in range(0, len(stage) - 1, 2):
                                vv = bt.tile([C, ML], f32, name=f"pv{gen}{i}")
                                nc.vector.tensor_add(vv[:], stage[i][:],
                                                     stage[i + 1][:])
                                nxt.append(vv)
                            if len(stage) % 2:
                                nxt.append(stage[-1])
                            stage = nxt
                            gen += 1
                        usum = stage[0]
                        btT = bt.tile([C, ML], f32, name="btT")
                        nc.vector.tensor_add(btT[:], usum[:], et_sb[:])
                        if it < NPOST - 1:
                            btTb = bt.tile([C, ML], bf16, name="btTb")
                            nc.scalar.activation(btTb[:], btT[:], AF.Copy)
                            psum_ym = ps5m.tile([P, MT, C], f32,
                                                name="psum_ym")
                            for m in range(MT):
                                nc.tensor.matmul(
                                    psum_ym[:, m, :],
                                    btTb[:, m * P:(m + 1) * P], h_sb[:],
                                    start=True, stop=True)
                            yloc = bt.tile([P, MT, C], bf16, name="yloc")
                            nc.scalar.activation(yloc[:], psum_ym[:], AF.Copy)
                            yloc_dram = dram_pool.tile([ML, C], bf16,
                                                       name=f"ylocd{it}")
                            nc.sync.dma_start(
                                yloc_dram.rearrange("(mt p) c -> p mt c", p=P),
                                yloc[:])
                            yfull = dram_pool.tile([NK, C], bf16,
                                                   name=f"yfullKM{it + 1}",
                                                   addr_space="Shared")
                            nc.gpsimd.collective_compute(
                                "AllGather", mybir.AluOpType.bypass,
                                replica_groups=rg,
                                ins=[yloc_dram[:].opt()], outs=[yfull[:].opt()])
                            nc.sync.dma_start(
                                y_sb[:],
                                yfull.rearrange("(kt p) c -> p kt c", p=P))
                        else:
                            outT_sb = bt.tile([C, ML], f32, name="outsb")
                            nc.vector.tensor_scalar_add(outT_sb[:], btT[:],
                                                        1.0 / C)
                            nc.sync.dma_start(outT[:], outT_sb[:])
            else:
                # truncated build: still write the output tensor
                with tc.tile_pool(name="dummy", bufs=1) as dummy:
                    dpad = dummy.tile([C, ML], f32)
                    nc.gpsimd.memset(dpad[:], 0.0)
                    nc.sync.dma_start(outT[:], dpad[:])

            adjr_cm.__exit__(None, None, None)

    nc.compile()
    return nc


def _get_compiled():
    if "nc" not in _CACHE:
        _CACHE["nc"] = _build_and_compile()
    return _CACHE["nc"]


def _prep_inputs(raw_adj, normed_adj, features, W1, b1, W2, b2, H):
    bf = ml_dtypes.bfloat16
    f8 = ml_dtypes.float8_e3m4
    w1b = np.ascontiguousarray(W1).astype(bf)
    w2b = np.ascontiguousarray(W2).astype(bf)
    hb = np.ascontiguousarray(H).astype(bf)
    b1c = np.asarray(b1, dtype=np.float32).reshape(HID, 1).copy()
    b2col = np.asarray(b2, dtype=np.float32).reshape(C, 1).copy()
    xT = np.ascontiguousarray(features.T).astype(np.float32)
    in_maps = []
    for r in range(RANKS):
        r0 = r * ML
        r1 = min(r0 + ML, NREAL)
        nr = r1 - r0
        xtl = np.zeros((F, ML), dtype=bf)
        xtl[:, :nr] = xT[:, r0:r1].astype(bf)
        an = np.zeros((NK, ML), dtype=f8)
        an[:NREAL, :nr] = (
            np.ascontiguousarray(normed_adj[r0:r1].T) * SN).astype(f8)
        ar = np.zeros((NK, ML), dtype=f8)
        ar[:NREAL, :nr] = (
            np.ascontiguousarray(raw_adj[r0:r1].T) * SR).astype(f8)
        in_maps.append({
            "adjTn": an, "adjTr": ar, "xTl": xtl, "w1": w1b, "w2": w2b,
            "hm": hb, "b1": b1c, "b2c": b2col,
        })
    return in_maps


def run_on_device(in_maps, trace=False):
    from concourse import bass_utils
    nc = _get_compiled()
    return bass_utils.run_bass_kernel_spmd(
        nc, in_maps, core_ids=list(range(RANKS)), trace=trace)


def kernel(raw_adj, normed_adj, features, y_onehot, train_mask,
           W1, b1, W2, b2, H):
    in_maps = _prep_inputs(np.asarray(raw_adj), np.asarray(normed_adj),
                           np.asarray(features), np.asarray(W1),
                           np.asarray(b1), np.asarray(W2), np.asarray(b2),
                           np.asarray(H))
    res = run_on_device(in_maps)
    parts = []
    for r in range(RANKS):
        o = np.asarray(res.results[r]["outT"], dtype=np.float32)  # [C, ML]
        parts.append(o.T)
    full = np.concatenate(parts, axis=0)[:NREAL]
    return np.ascontiguousarray(full).astype(np.float32)
